# revision 41
# baseline (speedup 1.0000x reference)
"""Trainium2 Bass kernel for DiagonalSSMLayer.

Math: y = C_w @ h + D*u  where  h[l] = lam*h[l-1] + (B_w @ u)[l]  (per state
channel, lam = sigmoid(log_lambda)).  The reference computes the causal
exponential-decay convolution via FFT; here it is the exact linear recurrence,
done with the DVE's native tensor_tensor_scan.

Sharding: 8 cores = (batch b in 0..3) x (sequence half s in 0..1).
Each core gets u[b, s*2048:(s+1)*2048, :] transposed to [D=1024, 2048] so the
contraction dim d sits on SBUF partitions for both GEMMs (out = lhsT.T @ rhs
contracts over the partition dim).  All HBM traffic is bf16.

Cross-half carry: second-half cores prepend a HALO of the last `HALO`
positions of the first half; the halo scan collapses to one weighted-sum STT
(accum_out) per n-tile, reconstructing the incoming state up to lam^HALO
(<= 0.08 of the carried state, ~1e-3 end-to-end).  First-half cores get a
zero halo, keeping the program uniform across cores (SPMD).

Schedule notes (from perfetto traces):
 - A single HWDGE generator streams inputs at only ~150 GB/s, which starved
   the PE (and a starved PE drops from 2.4 GHz to the 1.2 GHz p-state,
   doubling matmul time until ~5us of continuous work).  So every input
   tensor is split k-wise across BOTH HWDGE engines (sync + scalar), all
   triggers hoisted, in consumption order: BwT, halo, chunk1..2, CwT
   (GEMM2-only), chunk3..4.  The halo GEMM must run FIRST: deferring it
   (to start GEMM1 sooner) lands the carry so late that every GEMM2 waits
   on its scan and the p-state collapses (measured 61.7us vs 56.9us).
 - The y materialize alternates per k-tile between a fused
   scalar_tensor_tensor (u*D + y_ps, 560ns) on the DVE and a plain
   PSUM->SBUF copy on the otherwise-idle Act engine (823ns); the D*u term
   for the Act k-tiles is added host-side.  One y-materialize engine alone
   (560ns cadence vs the 432ns GEMM2 matmul pair) stalled the PE ~220ns
   per k-tile.
 - PE warmup matmuls bridge the initial DMA wait; a second short bridge of
   cheap 256-free warmups sits between the halo GEMM and GEMM1(chunk1) to
   cover chunk1's DMA arrival without a p-state-resetting idle gap.
 - A dummy Act copy preloads the activation table during the DMA wait.
 - y leaves per chunk as one 8KB/partition DMA on the (input-free by then)
   sync queue; the last chunk streams per-k, pair-swapped so the stream
   ends on the faster DVE STT.
"""

import numpy as np
import ml_dtypes

BF16 = ml_dtypes.bfloat16

B, L, DM, NS = 4, 4096, 1024, 256
HALF = L // 2          # 2048 sequence positions per core
NCORES = 8
LC = 512               # l-chunk (matmul free dim / scan chunk)
NLC = HALF // LC       # 4 main chunks
HALO = 256
KT = DM // 128         # 8 k-tiles (contraction over d)
NT = NS // 128         # 2 n-tiles (state channels)

_CACHE = {}


def _build(warm=12, bridge=12):
    from concourse import bacc, tile, mybir

    MULT = mybir.AluOpType.mult
    ADD = mybir.AluOpType.add
    f32 = mybir.dt.float32
    bf16 = mybir.dt.bfloat16

    nc = bacc.Bacc("TRN2", target_bir_lowering=False, debug=False,
                   num_devices=NCORES)

    # chunk-major u so each chunk's DMA is one contiguous run per partition.
    # BwT and the halo are packed into one "head" tensor ([k: BwT_k | uH_k])
    # so the front of the stream is one big-descriptor transfer per engine.
    head_d = nc.dram_tensor("head", [128, KT, NS + HALO], bf16,
                            kind="ExternalInput").ap()
    uT_d = nc.dram_tensor("uT", [128, NLC, KT, LC], bf16, kind="ExternalInput").ap()
    CwT_d = nc.dram_tensor("CwT", [128, NT, DM], bf16, kind="ExternalInput").ap()
    # packed small params: cols [0..NT) = lam per n-tile, [NT..NT+KT) = D per k
    par_d = nc.dram_tensor("params", [128, NT + KT], f32, kind="ExternalInput").ap()
    # lam^(HALO-1-j) weights: the halo only needs its FINAL state, so the
    # halo scan collapses to one weighted-sum STT (accum_out) per n-tile
    lph_d = nc.dram_tensor("lampowH", [128, NT * HALO], bf16, kind="ExternalInput").ap()
    yT_d = nc.dram_tensor("yT", [128, NLC, KT, LC], bf16, kind="ExternalOutput").ap()

    kh = KT // 2

    with tile.TileContext(nc) as tc:
        with tc.tile_pool(name="const", bufs=1) as cpool, \
             tc.tile_pool(name="u", bufs=1) as upool, \
             tc.tile_pool(name="h", bufs=1) as hpool, \
             tc.tile_pool(name="y", bufs=3) as ypool, \
             tc.tile_pool(name="bu_ps", bufs=3, space="PSUM") as bupool, \
             tc.tile_pool(name="y_ps", bufs=5, space="PSUM") as yppool:

            # memset FIRST on gpsimd (before its DMA triggers) so the warmup
            # matmuls can start as early as possible
            warm_sb = cpool.tile([128, 512], bf16, name="warm")
            nc.gpsimd.memset(warm_sb[:], 1.0)
            act_dummy = cpool.tile([128, 1], f32, name="actdum")
            head3 = cpool.tile([128, KT, NS + HALO], bf16, name="head")
            BwT_sb = [head3[:, k, 0:NS] for k in range(KT)]
            par3 = cpool.tile([128, NT + KT], f32, name="par")
            nc.gpsimd.dma_start(out=par3[:], in_=par_d[:, :])
            lamv_sb = [par3[:, n:n + 1] for n in range(NT)]
            dvec_sb = [par3[:, NT + k:NT + k + 1] for k in range(KT)]
            lph3 = cpool.tile([128, NT * HALO], bf16, name="lph")
            nc.gpsimd.dma_start(out=lph3[:], in_=lph_d[:, :])
            carry = [cpool.tile([128, 1], f32, name=f"carry{n}") for n in range(NT)]
            CwT3 = cpool.tile([128, NT, DM], bf16, name="cw")
            CwT_sb = [CwT3[:, n, :] for n in range(NT)]

            lam_sb = [cpool.tile([128, LC], f32, name=f"lam{n}") for n in range(NT)]
            for n in range(NT):
                nc.vector.memset(lam_sb[n][:], 1.0)
                nc.vector.tensor_scalar_mul(lam_sb[n][:], lam_sb[n][:],
                                            lamv_sb[n])

            # ---- PE warmup: dummy matmuls raise the HAM clock out of the
            # low p-state while the first inputs are still in flight
            warm_ps = yppool.tile([128, LC], f32, tag="y")
            for w in range(warm):
                nc.tensor.matmul(warm_ps[:], warm_sb[:, 0:128], warm_sb[:],
                                 start=(w == 0), stop=(w == warm - 1))

            hr = [hpool.tile([128, HALF], bf16, name=f"hr_{n}") for n in range(NT)]
            hh = [hpool.tile([128, HALO], bf16, name=f"hh_{n}") for n in range(NT)]

            uC_sb = [upool.tile([128, KT, LC], bf16, name=f"uc{c}")
                     for c in range(NLC)]

            # ---- input stream: k-halves split across the two HWDGE
            # generators (sync gets k0-3, scalar k4-7), all triggers hoisted
            # so neither queue ever blocks an input behind a y-out wait
            def both(dst, src):
                nc.sync.dma_start(out=dst[:, 0:kh], in_=src[:, 0:kh])
                nc.scalar.dma_start(out=dst[:, kh:KT], in_=src[:, kh:KT])

            both(head3, head_d)
            both(uC_sb[0], uT_d[:, 0])
            both(uC_sb[1], uT_d[:, 1])
            nc.sync.dma_start(out=CwT3[:, 0:1], in_=CwT_d[:, 0:1])
            nc.scalar.dma_start(out=CwT3[:, 1:2], in_=CwT_d[:, 1:2])
            both(uC_sb[2], uT_d[:, 2])
            both(uC_sb[3], uT_d[:, 3])

            # preload the Act table during the DMA wait (first InstActivation
            # pays a ~1.5us table load)
            nc.scalar.copy(act_dummy[:], warm_sb[:, 0:1])

            # ---- halo GEMM -> carry
            for n in range(NT):
                buh = bupool.tile([128, HALO], f32, tag="bu")
                for k in range(KT):
                    nc.tensor.matmul(buh[:],
                                     BwT_sb[k][:, n * 128:(n + 1) * 128],
                                     head3[:, k, NS:NS + HALO],
                                     start=(k == 0), stop=(k == KT - 1))
                # carry = sum_j bu[j] * lam^(HALO-1-j): one STT with a
                # row-sum accumulator instead of a full (2x slower) scan
                nc.vector.scalar_tensor_tensor(
                    hh[n][:], buh[:], 1.0,
                    lph3[:, n * HALO:(n + 1) * HALO],
                    MULT, MULT, accum_out=carry[n][:])

            # ---- bridge warmups: cheap 256-free matmuls covering chunk1's
            # DMA arrival so the PE never idles (idle resets the p-state)
            if bridge:
                for w in range(bridge):
                    nc.tensor.matmul(warm_ps[:, 0:HALO], warm_sb[:, 0:128],
                                     warm_sb[:, 0:HALO],
                                     start=(w == 0), stop=(w == bridge - 1))

            # ---- main chunks: GEMM1 -> scan -> GEMM2 -> y out.
            # GEMM2/y-mat run one chunk behind the scan chain so the next
            # scan never queues behind y work on the in-order DVE.
            def gemm2(c):
                # y streams out per k-PAIR (2KB/partition) for EVERY chunk,
                # all triggers on the otherwise-idle sync queue: coarse
                # whole-chunk DMAs made the in-order queue park on one big
                # wait and bunch every later trigger after the last matmul
                o = c * LC
                y_sb = ypool.tile([128, KT, LC], bf16, tag="ysb")
                for k in range(KT):
                    y_ps = yppool.tile([128, LC], f32, tag="y")
                    for n in range(NT):
                        nc.tensor.matmul(y_ps[:],
                                         CwT_sb[n][:, k * 128:(k + 1) * 128],
                                         hr[n][:, o:o + LC],
                                         start=(n == 0), stop=(n == NT - 1))
                    # y materialize alternates DVE (fused u*D + y_ps) and
                    # Act (plain copy; D*u for these k added host-side)
                    if k % 2 == 0:
                        nc.vector.scalar_tensor_tensor(
                            y_sb[:, k, :], uC_sb[c][:, k, :],
                            dvec_sb[k], y_ps[:], MULT, ADD)
                        if c == NLC - 1 and k == KT - 2:
                            # very last pair ships as two singles so k6's
                            # data doesn't wait for k7's Act copy
                            nc.sync.dma_start(out=yT_d[:, c, k, :],
                                              in_=y_sb[:, k, :])
                    else:
                        nc.scalar.copy(y_sb[:, k, :], y_ps[:])
                        if c == NLC - 1 and k == KT - 1:
                            nc.sync.dma_start(out=yT_d[:, c, k, :],
                                              in_=y_sb[:, k, :])
                        else:
                            nc.sync.dma_start(out=yT_d[:, c, k - 1:k + 1, :],
                                              in_=y_sb[:, k - 1:k + 1, :])

            for c in range(NLC):
                o = c * LC
                for n in range(NT):
                    bu_ps = bupool.tile([128, LC], f32, tag="bu")
                    for k in range(KT):
                        nc.tensor.matmul(bu_ps[:],
                                         BwT_sb[k][:, n * 128:(n + 1) * 128],
                                         uC_sb[c][:, k, :],
                                         start=(k == 0), stop=(k == KT - 1))
                    init = (carry[n][:] if c == 0
                            else hr[n][:, o - 1:o])
                    nc.vector.tensor_tensor_scan(
                        hr[n][:, o:o + LC],
                        lam_sb[n][:], bu_ps[:], init, MULT, ADD)
                if c > 0:
                    gemm2(c - 1)
            gemm2(NLC - 1)

    nc.compile()
    return nc


def _sigmoid(x):
    return 1.0 / (1.0 + np.exp(-x))


def kernel(u, log_lambda, B_w, C_w, D):
    from concourse.bass_utils import run_bass_kernel_spmd

    if "nc" not in _CACHE:
        _CACHE["nc"] = _build()
    nc = _CACHE["nc"]

    u = np.asarray(u, dtype=np.float32)
    lam = _sigmoid(np.asarray(log_lambda, dtype=np.float64))
    # p-major layouts: [128, KT, ...] so one dma_start covers all k-tiles
    BwT = np.ascontiguousarray(
        np.asarray(B_w, np.float32).T.reshape(KT, 128, NS).transpose(1, 0, 2)
    ).astype(BF16)
    CwT = np.ascontiguousarray(
        np.asarray(C_w, np.float32).T.reshape(NT, 128, DM).transpose(1, 0, 2)
    ).astype(BF16)
    params = np.empty((128, NT + KT), dtype=np.float32)
    params[:, :NT] = lam.reshape(NT, 128).T.astype(np.float32)
    params[:, NT:] = np.asarray(D, np.float32).reshape(KT, 128).T
    # lam^(HALO-1-j) [128, NT*HALO]: partition p, n-tile-major columns
    lph = (lam[:, None] ** np.arange(HALO - 1, -1, -1)[None, :])
    lampowH = np.ascontiguousarray(
        lph.reshape(NT, 128, HALO).transpose(1, 0, 2)
    ).astype(BF16).reshape(128, NT * HALO)

    zero_halo = np.zeros((128, KT, HALO), dtype=BF16)
    in_maps = []
    for core in range(NCORES):
        b, s = core // 2, core % 2
        if s == 1:
            uH = np.ascontiguousarray(
                u[b, HALF - HALO:HALF, :].T.reshape(KT, 128, HALO)
                .transpose(1, 0, 2)).astype(BF16)
        else:
            uH = zero_halo
        # head = per k: [BwT_k | uH_k] so the stream front is one transfer
        head = np.concatenate([BwT, uH], axis=2)
        # [p, chunk, k, l] with d = k*128 + p, col = chunk*LC + l
        uTh = np.ascontiguousarray(
            u[b, s * HALF:(s + 1) * HALF, :].T
            .reshape(KT, 128, NLC, LC).transpose(1, 2, 0, 3)
        ).astype(BF16)
        in_maps.append({
            "head": head,
            "uT": uTh,
            "CwT": CwT,
            "params": params,
            "lampowH": lampowH,
        })
    _CACHE["in_maps"] = in_maps

    def _run():
        return run_bass_kernel_spmd(nc, in_maps, core_ids=list(range(NCORES)))

    try:
        res = _run()
    except Exception:
        # a previously failed execution can wedge the backend; reset + retry
        try:
            import ctypes, jax
            jax.devices()
            lib = ctypes.CDLL("/opt/axon/libaxon_pjrt.so")
            lib.axon_reset.restype = ctypes.c_int64
            lib.axon_reset()
        except Exception:
            pass
        res = _run()

    y = np.empty((B, L, DM), dtype=np.float32)
    for core in range(NCORES):
        b, s = core // 2, core % 2
        yT = res.results[core]["yT"].astype(np.float32)   # [128, NLC, KT, LC]
        y[b, s * HALF:(s + 1) * HALF, :] = (
            yT.transpose(2, 0, 1, 3).reshape(DM, HALF).T)
    # host epilogue: D*u for the k-tiles whose y left the device as a plain
    # PSUM copy (odd k, i.e. d in [k*128,(k+1)*128) for odd k)
    Dm = np.asarray(D, np.float32).copy().reshape(KT, 128)
    Dm[0::2] = 0.0
    y += Dm.reshape(DM) * u
    return y


# revision 42
# speedup vs baseline: 1.1410x; 1.1410x over previous
"""Trainium2 Bass kernel for DiagonalSSMLayer.

Math: y = C_w @ h + D*u  where  h[l] = lam*h[l-1] + (B_w @ u)[l]  (per state
channel, lam = sigmoid(log_lambda)).  The reference computes the causal
exponential-decay convolution via FFT; here it is the exact linear recurrence,
done with the DVE's native tensor_tensor_scan.

Sharding: 8 cores = (batch b in 0..3) x (sequence half s in 0..1).
Each core gets u[b, s*2048:(s+1)*2048, :] transposed to [D=1024, 2048] so the
contraction dim d sits on SBUF partitions for both GEMMs (out = lhsT.T @ rhs
contracts over the partition dim).  All HBM traffic is bf16.

Cross-half carry: second-half cores prepend a HALO of the last `HALO`
positions of the first half; the halo scan collapses to one weighted-sum STT
(accum_out) per n-tile, reconstructing the incoming state up to lam^HALO
(<= 0.08 of the carried state, ~1e-3 end-to-end).  First-half cores get a
zero halo, keeping the program uniform across cores (SPMD).

Schedule notes (from perfetto traces):
 - A single HWDGE generator streams inputs at only ~150 GB/s, which starved
   the PE (and a starved PE drops from 2.4 GHz to the 1.2 GHz p-state,
   doubling matmul time until ~5us of continuous work).  So every input
   tensor is split k-wise across BOTH HWDGE engines (sync + scalar), all
   triggers hoisted, in consumption order: BwT, halo, chunk1..2, CwT
   (GEMM2-only), chunk3..4.  The halo GEMM must run FIRST: deferring it
   (to start GEMM1 sooner) lands the carry so late that every GEMM2 waits
   on its scan and the p-state collapses (measured 61.7us vs 56.9us).
 - The y materialize alternates per k-tile between a fused
   scalar_tensor_tensor (u*D + y_ps, 560ns) on the DVE and a plain
   PSUM->SBUF copy on the otherwise-idle Act engine (823ns); the D*u term
   for the Act k-tiles is added host-side.  One y-materialize engine alone
   (560ns cadence vs the 432ns GEMM2 matmul pair) stalled the PE ~220ns
   per k-tile.
 - PE warmup matmuls bridge the initial DMA wait; a second short bridge of
   cheap 256-free warmups sits between the halo GEMM and GEMM1(chunk1) to
   cover chunk1's DMA arrival without a p-state-resetting idle gap.
 - A dummy Act copy preloads the activation table during the DMA wait.
 - y leaves per chunk as one 8KB/partition DMA on the (input-free by then)
   sync queue; the last chunk streams per-k, pair-swapped so the stream
   ends on the faster DVE STT.
"""

import numpy as np
import ml_dtypes

BF16 = ml_dtypes.bfloat16

B, L, DM, NS = 4, 4096, 1024, 256
HALF = L // 2          # 2048 sequence positions per core
NCORES = 8
LC = 512               # l-chunk (matmul free dim / scan chunk)
NLC = HALF // LC       # 4 main chunks
HALO = 256
KT = DM // 128         # 8 k-tiles (contraction over d)
NT = NS // 128         # 2 n-tiles (state channels)

_CACHE = {}


def _build(warm=12, bridge=12):
    from concourse import bacc, tile, mybir

    MULT = mybir.AluOpType.mult
    ADD = mybir.AluOpType.add
    f32 = mybir.dt.float32
    bf16 = mybir.dt.bfloat16

    nc = bacc.Bacc("TRN2", target_bir_lowering=False, debug=False,
                   num_devices=NCORES)

    # chunk-major u so each chunk's DMA is one contiguous run per partition.
    # BwT and the halo are packed into one "head" tensor ([k: BwT_k | uH_k])
    # so the front of the stream is one big-descriptor transfer per engine.
    head_d = nc.dram_tensor("head", [128, KT, NS + HALO], bf16,
                            kind="ExternalInput").ap()
    uT_d = nc.dram_tensor("uT", [128, NLC, KT, LC], bf16, kind="ExternalInput").ap()
    CwT_d = nc.dram_tensor("CwT", [128, NT, DM], bf16, kind="ExternalInput").ap()
    # packed small params: cols [0..NT) = lam per n-tile, [NT..NT+KT) = D per k
    par_d = nc.dram_tensor("params", [128, NT + KT], f32, kind="ExternalInput").ap()
    # lam^(HALO-1-j) weights: the halo only needs its FINAL state, so the
    # halo scan collapses to one weighted-sum STT (accum_out) per n-tile
    lph_d = nc.dram_tensor("lampowH", [128, NT * HALO], bf16, kind="ExternalInput").ap()
    yT_d = nc.dram_tensor("yT", [128, NLC, KT, LC], bf16, kind="ExternalOutput").ap()

    kh = KT // 2

    with tile.TileContext(nc) as tc:
        with tc.tile_pool(name="const", bufs=1) as cpool, \
             tc.tile_pool(name="u", bufs=1) as upool, \
             tc.tile_pool(name="h", bufs=1) as hpool, \
             tc.tile_pool(name="y", bufs=3) as ypool, \
             tc.tile_pool(name="bu_ps", bufs=3, space="PSUM") as bupool, \
             tc.tile_pool(name="y_ps", bufs=5, space="PSUM") as yppool:

            # memset FIRST on gpsimd (before its DMA triggers) so the warmup
            # matmuls can start as early as possible
            warm_sb = cpool.tile([128, 512], bf16, name="warm")
            nc.gpsimd.memset(warm_sb[:], 1.0)
            act_dummy = cpool.tile([128, 1], f32, name="actdum")
            head3 = cpool.tile([128, KT, NS + HALO], bf16, name="head")
            BwT_sb = [head3[:, k, 0:NS] for k in range(KT)]
            par3 = cpool.tile([128, NT + KT], f32, name="par")
            nc.gpsimd.dma_start(out=par3[:], in_=par_d[:, :])
            lamv_sb = [par3[:, n:n + 1] for n in range(NT)]
            dvec_sb = [par3[:, NT + k:NT + k + 1] for k in range(KT)]
            lph3 = cpool.tile([128, NT * HALO], bf16, name="lph")
            nc.gpsimd.dma_start(out=lph3[:], in_=lph_d[:, :])
            carry = [cpool.tile([128, 1], f32, name=f"carry{n}") for n in range(NT)]
            CwT3 = cpool.tile([128, NT, DM], bf16, name="cw")
            CwT_sb = [CwT3[:, n, :] for n in range(NT)]

            lam_sb = [cpool.tile([128, LC], f32, name=f"lam{n}") for n in range(NT)]
            for n in range(NT):
                nc.vector.memset(lam_sb[n][:], 1.0)
                nc.vector.tensor_scalar_mul(lam_sb[n][:], lam_sb[n][:],
                                            lamv_sb[n])

            # ---- PE warmup: dummy matmuls raise the HAM clock out of the
            # low p-state while the first inputs are still in flight
            warm_ps = yppool.tile([128, LC], f32, tag="y")
            for w in range(warm):
                nc.tensor.matmul(warm_ps[:], warm_sb[:, 0:128], warm_sb[:],
                                 start=(w == 0), stop=(w == warm - 1))

            hr = [hpool.tile([128, HALF], bf16, name=f"hr_{n}") for n in range(NT)]
            hh = [hpool.tile([128, HALO], bf16, name=f"hh_{n}") for n in range(NT)]

            uC_sb = [upool.tile([128, KT, LC], bf16, name=f"uc{c}")
                     for c in range(NLC)]

            # ---- input stream: k-halves split across the two HWDGE
            # generators (sync gets k0-3, scalar k4-7), all triggers hoisted
            # so neither queue ever blocks an input behind a y-out wait
            def both(dst, src):
                nc.sync.dma_start(out=dst[:, 0:kh], in_=src[:, 0:kh])
                nc.scalar.dma_start(out=dst[:, kh:KT], in_=src[:, kh:KT])

            both(head3, head_d)
            both(uC_sb[0], uT_d[:, 0])
            both(uC_sb[1], uT_d[:, 1])
            nc.sync.dma_start(out=CwT3[:, 0:1], in_=CwT_d[:, 0:1])
            nc.scalar.dma_start(out=CwT3[:, 1:2], in_=CwT_d[:, 1:2])
            both(uC_sb[2], uT_d[:, 2])
            both(uC_sb[3], uT_d[:, 3])

            # preload the Act table during the DMA wait (first InstActivation
            # pays a ~1.5us table load)
            nc.scalar.copy(act_dummy[:], warm_sb[:, 0:1])

            # ---- halo GEMM -> carry
            for n in range(NT):
                buh = bupool.tile([128, HALO], f32, tag="bu")
                for k in range(KT):
                    nc.tensor.matmul(buh[:],
                                     BwT_sb[k][:, n * 128:(n + 1) * 128],
                                     head3[:, k, NS:NS + HALO],
                                     start=(k == 0), stop=(k == KT - 1))
                # carry = sum_j bu[j] * lam^(HALO-1-j): one STT with a
                # row-sum accumulator instead of a full (2x slower) scan
                nc.vector.scalar_tensor_tensor(
                    hh[n][:], buh[:], 1.0,
                    lph3[:, n * HALO:(n + 1) * HALO],
                    MULT, MULT, accum_out=carry[n][:])

            # ---- bridge warmups: cheap 256-free matmuls covering chunk1's
            # DMA arrival so the PE never idles (idle resets the p-state)
            if bridge:
                for w in range(bridge):
                    nc.tensor.matmul(warm_ps[:, 0:HALO], warm_sb[:, 0:128],
                                     warm_sb[:, 0:HALO],
                                     start=(w == 0), stop=(w == bridge - 1))

            # ---- main chunks: GEMM1 -> scan -> GEMM2 -> y out.
            # GEMM2/y-mat run one chunk behind the scan chain so the next
            # scan never queues behind y work on the in-order DVE.
            def gemm2(c):
                # y streams out per k-PAIR (2KB/partition) for EVERY chunk,
                # all triggers on the otherwise-idle sync queue: coarse
                # whole-chunk DMAs made the in-order queue park on one big
                # wait and bunch every later trigger after the last matmul
                o = c * LC
                y_sb = ypool.tile([128, KT, LC], bf16, tag="ysb")
                for k in range(KT):
                    y_ps = yppool.tile([128, LC], f32, tag="y")
                    for n in range(NT):
                        nc.tensor.matmul(y_ps[:],
                                         CwT_sb[n][:, k * 128:(k + 1) * 128],
                                         hr[n][:, o:o + LC],
                                         start=(n == 0), stop=(n == NT - 1))
                    # y materialize alternates DVE (fused u*D + y_ps) and
                    # Act (plain copy; D*u for these k added host-side)
                    if k % 2 == 0:
                        nc.vector.scalar_tensor_tensor(
                            y_sb[:, k, :], uC_sb[c][:, k, :],
                            dvec_sb[k], y_ps[:], MULT, ADD)
                    else:
                        nc.scalar.copy(y_sb[:, k, :], y_ps[:])
                        nc.sync.dma_start(out=yT_d[:, c, k - 1:k + 1, :],
                                          in_=y_sb[:, k - 1:k + 1, :])

            for c in range(NLC):
                o = c * LC
                for n in range(NT):
                    bu_ps = bupool.tile([128, LC], f32, tag="bu")
                    for k in range(KT):
                        nc.tensor.matmul(bu_ps[:],
                                         BwT_sb[k][:, n * 128:(n + 1) * 128],
                                         uC_sb[c][:, k, :],
                                         start=(k == 0), stop=(k == KT - 1))
                    init = (carry[n][:] if c == 0
                            else hr[n][:, o - 1:o])
                    nc.vector.tensor_tensor_scan(
                        hr[n][:, o:o + LC],
                        lam_sb[n][:], bu_ps[:], init, MULT, ADD)
                if c > 0:
                    gemm2(c - 1)
            gemm2(NLC - 1)

    nc.compile()
    return nc


def _sigmoid(x):
    return 1.0 / (1.0 + np.exp(-x))


def kernel(u, log_lambda, B_w, C_w, D):
    from concourse.bass_utils import run_bass_kernel_spmd

    if "nc" not in _CACHE:
        _CACHE["nc"] = _build()
    nc = _CACHE["nc"]

    u = np.asarray(u, dtype=np.float32)
    lam = _sigmoid(np.asarray(log_lambda, dtype=np.float64))
    # p-major layouts: [128, KT, ...] so one dma_start covers all k-tiles
    BwT = np.ascontiguousarray(
        np.asarray(B_w, np.float32).T.reshape(KT, 128, NS).transpose(1, 0, 2)
    ).astype(BF16)
    CwT = np.ascontiguousarray(
        np.asarray(C_w, np.float32).T.reshape(NT, 128, DM).transpose(1, 0, 2)
    ).astype(BF16)
    params = np.empty((128, NT + KT), dtype=np.float32)
    params[:, :NT] = lam.reshape(NT, 128).T.astype(np.float32)
    params[:, NT:] = np.asarray(D, np.float32).reshape(KT, 128).T
    # lam^(HALO-1-j) [128, NT*HALO]: partition p, n-tile-major columns
    lph = (lam[:, None] ** np.arange(HALO - 1, -1, -1)[None, :])
    lampowH = np.ascontiguousarray(
        lph.reshape(NT, 128, HALO).transpose(1, 0, 2)
    ).astype(BF16).reshape(128, NT * HALO)

    zero_halo = np.zeros((128, KT, HALO), dtype=BF16)
    in_maps = []
    for core in range(NCORES):
        b, s = core // 2, core % 2
        if s == 1:
            uH = np.ascontiguousarray(
                u[b, HALF - HALO:HALF, :].T.reshape(KT, 128, HALO)
                .transpose(1, 0, 2)).astype(BF16)
        else:
            uH = zero_halo
        # head = per k: [BwT_k | uH_k] so the stream front is one transfer
        head = np.concatenate([BwT, uH], axis=2)
        # [p, chunk, k, l] with d = k*128 + p, col = chunk*LC + l
        uTh = np.ascontiguousarray(
            u[b, s * HALF:(s + 1) * HALF, :].T
            .reshape(KT, 128, NLC, LC).transpose(1, 2, 0, 3)
        ).astype(BF16)
        in_maps.append({
            "head": head,
            "uT": uTh,
            "CwT": CwT,
            "params": params,
            "lampowH": lampowH,
        })
    _CACHE["in_maps"] = in_maps

    def _run():
        return run_bass_kernel_spmd(nc, in_maps, core_ids=list(range(NCORES)))

    try:
        res = _run()
    except Exception:
        # a previously failed execution can wedge the backend; reset + retry
        try:
            import ctypes, jax
            jax.devices()
            lib = ctypes.CDLL("/opt/axon/libaxon_pjrt.so")
            lib.axon_reset.restype = ctypes.c_int64
            lib.axon_reset()
        except Exception:
            pass
        res = _run()

    y = np.empty((B, L, DM), dtype=np.float32)
    for core in range(NCORES):
        b, s = core // 2, core % 2
        yT = res.results[core]["yT"].astype(np.float32)   # [128, NLC, KT, LC]
        y[b, s * HALF:(s + 1) * HALF, :] = (
            yT.transpose(2, 0, 1, 3).reshape(DM, HALF).T)
    # host epilogue: D*u for the k-tiles whose y left the device as a plain
    # PSUM copy (odd k, i.e. d in [k*128,(k+1)*128) for odd k)
    Dm = np.asarray(D, np.float32).copy().reshape(KT, 128)
    Dm[0::2] = 0.0
    y += Dm.reshape(DM) * u
    return y
